# revision 8
# baseline (speedup 1.0000x reference)
"""Trainium2 Bass kernel for nn_ConstLoss_22746146800082 (fp8 factorized).

loss = mean_{i != j} (Cq[i,j] - Ck[i,j])^2 with Cx the pairwise cosine matrix
of feat_x (N=4096, D=1024).  The Normalize/cosine eps terms cancel, so Cx is
the cosine matrix of the raw rows and the diagonal of Cq - Ck is ~0.

Factorization: ||Cq - Ck||_F^2 = ||Aqq||^2 + ||Akk||^2 - 2 ||Aqk||^2 with the
feature-space Grams Aqq = Qh^T Qh, Akk = Kh^T Kh, Aqk = Kh^T Qh of the
row-normalized features (1024x1024 each).

Host prep: rows are normalized, scaled by 32 and quantized to fp8e4 (e4m3) on
the host, so the device does no normalization, no collective, and the
stationary matmul operand is just a column slice of the streamed tile.

Sharding: core c owns feature block c (128 of 1024 output rows per Gram).
Each core's inputs have their columns ROTATED left by 128*c, so every core
runs the identical program.  sq ships Q blocks {c..c+3} (local [0:512)), sk
ships K blocks {c..c+4} (local [0:640)).  Per core: qq = Q_c^T Q_{c+1..c+3}
(block distances 1-3; summed over cores that covers every off-diagonal pair
except distance 4), kk likewise, and qk uses TWO K stationaries (K_c and
K_{c+4}) against the 4 Q moving blocks, which tiles all 64 (K_i, Q_j) block
pairs exactly once (j-i in {0..3} from K_c, {-4..-1} from K_{c+4}).  Matmuls
run as fp8 DoubleRow (2 chunks of 128 samples per instruction).  The device
returns raw per-region sums of squares; the host combines them as
2*acc + C0 + C4 with the diagonal-block and distance-4 Gram corrections
C0/C4 computed on the host from the same fp8 data (16% of the Gram FLOPs).
"""

import numpy as np

import concourse.bass as bass
import concourse.mybir as mybir
import concourse.tile as tile
from concourse.vector_clock import ScopedClock
from concourse.bass_utils import run_bass_kernel_spmd

N_CORES = 8
N = 4096
D = 1024
P = 128
NB = D // P               # 8 feature blocks
QW = 4 * P                # 512: sq ships Q blocks {c..c+3}
KW = 5 * P                # 640: sk ships K blocks {c..c+4}
MG = 4                    # chunks per DMA
ND = N // (P * MG)        # 8 DMA iterations
NG = N // (2 * P)         # 16 DoubleRow groups (2 chunks each)
GEN = NG // 2             # qk psum generation split
SCALE = 32.0

F32 = mybir.dt.float32
F8 = mybir.dt.float8e4
ACTF = mybir.ActivationFunctionType
DR = mybir.MatmulPerfMode.DoubleRow


class _TC(tile.TileContext):
    """TileContext whose kernel-tail drain splits its semaphore waits across
    preceding sync-engine NOPs: this container's walrus build rejects a Drain
    carrying more than one sync wait ("Too many sync wait commands")."""

    def _drain_and_barrier(self, tick_clock, wait_clock):
        nc = self.nc
        probe = nc.sync.nop(nofuse=True)
        wait_clock.add_sem_waits(
            probe.ins, ScopedClock({None: tick_clock.global_clock})
        )
        waits = list(probe.ins.sync_info.on_wait or []) if probe.ins.sync_info else []
        if probe.ins.sync_info is not None:
            probe.ins.sync_info.on_wait = waits[:1]
        for w in waits[1:]:
            n2 = nc.sync.nop(nofuse=True)
            n2.ins.sync_info = mybir.SyncInfo(on_wait=[w], on_update=[])
        nc.sync.drain()
        nc.all_engine_barrier()
        popped = nc._tile_sem_poison_stack.pop()
        assert popped is self._sem_poison
        nc.clear_and_free_semaphores(list(self.sems.allocated().values()))
        nc.all_engine_barrier()


MAX_WAITS_PER_INST = 1


def split_excess_waits(nc):
    """walrus (this build) rejects instructions carrying more than a couple
    of semaphore waits.  Hoist excess waits onto injected same-engine NOPs
    placed immediately before the offending instruction."""
    n = 0
    for f in nc.m.functions:
        for bb in f.blocks:
            insts = bb.instructions
            out = []
            changed = False
            for ins in insts:
                si = ins.sync_info
                waits = list(si.on_wait or []) if si is not None else []
                while len(waits) > MAX_WAITS_PER_INST:
                    take = waits[:MAX_WAITS_PER_INST]
                    waits = waits[MAX_WAITS_PER_INST:]
                    nop = mybir.InstNoOp(name=f"I-waitsplit-{n}", ins=[], outs=[])
                    n += 1
                    nop.engine = ins.engine
                    nop.sync_info = mybir.SyncInfo(on_wait=take, on_update=[])
                    out.append(nop)
                    changed = True
                if changed and si is not None:
                    si.on_wait = waits
                out.append(ins)
            if changed:
                bb.instructions = out
    return n


def build_program(sim_mode: bool = False):
    nc = bass.Bass(
        "TRN2", target_bir_lowering=False, debug=False, num_devices=N_CORES
    )
    sq = nc.dram_tensor("sq", [N, QW], F8, kind="ExternalInput").ap()
    sk = nc.dram_tensor("sk", [N, KW], F8, kind="ExternalInput").ap()
    out = nc.dram_tensor("out", [P, 3], F32, kind="ExternalOutput").ap()

    with _TC(nc) as tc:
        with (
            tc.tile_pool(name="stream", bufs=3) as stream,
            tc.tile_pool(name="fin", bufs=1) as fin,
            tc.tile_pool(name="psum", bufs=1, space="PSUM") as psum,
        ):
            accs = fin.tile([P, 3], F32)
            scr = fin.tile([P, 2, 512], F32)

            # One psum tile covering 6 of 8 banks, viewed as [P, bank, 512]:
            # bank 0 = qq [0:384), bank 1 = kk [0:384), banks 2-3 = qk
            # generation A (K_c and K_{c+4} stationary rows), banks 4-5 = qk
            # generation B.  Every accumulation region sits alone in its bank
            # so the whole-bank start_tensor_calc zeroing is safe.
            ps = psum.tile([P, 6, 512], F32, name="ps", tag="ps")

            # chunk schedule: 4-chunk DMAs mid-stream, 2-chunk (single-group)
            # DMAs at the end so the final transfer gates only one group of
            # matmuls
            sched = [4] * ((2 * NG - 4) // 4) + [2, 2]
            g0 = 0
            for mg in sched:
                c0 = 2 * g0  # first chunk of this DMA
                tq = stream.tile([P, mg, QW], F8, name="tq", tag="tq")
                tk = stream.tile([P, mg, KW], F8, name="tk", tag="tk")
                nc.sync.dma_start(
                    out=tk,
                    in_=bass.AP(
                        sk.tensor, sk.offset + c0 * P * KW,
                        [[KW, P], [P * KW, mg], [1, KW]],
                    ),
                )
                nc.sync.dma_start(
                    out=tq,
                    in_=bass.AP(
                        sq.tensor, sq.offset + c0 * P * QW,
                        [[QW, P], [P * QW, mg], [1, QW]],
                    ),
                )
                for h in range(mg // 2):
                    g = g0 + h
                    sl = slice(2 * h, 2 * h + 2)
                    lq = tq[:, sl, 0:P]      # [128, 2, 128] stationary Q_c
                    lk0 = tk[:, sl, 0:P]     # [128, 2, 128] stationary K_c
                    lk4 = tk[:, sl, 4 * P : 5 * P]  # stationary K_{c+4}
                    st = dict(
                        start=(g == 0), stop=(g == NG - 1),
                        perf_mode=DR, skip_group_check=True,
                    )
                    nc.tensor.matmul(ps[:, 1, 0:384], lhsT=lk0, rhs=tk[:, sl, P:QW], **st)
                    nc.tensor.matmul(ps[:, 0, 0:384], lhsT=lq, rhs=tq[:, sl, P:QW], **st)
                    qb = 2 if g < GEN else 4
                    stq = dict(
                        start=(g % GEN == 0), stop=(g % GEN == GEN - 1),
                        perf_mode=DR, skip_group_check=True,
                    )
                    nc.tensor.matmul(ps[:, qb, :], lhsT=lk0, rhs=tq[:, sl, 0:QW], **stq)
                    nc.tensor.matmul(ps[:, qb + 1, :], lhsT=lk4, rhs=tq[:, sl, 0:QW], **stq)
                g0 += mg // 2

                if g0 == GEN:
                    # qk generation A done: square it while the second half
                    # of the stream is still in flight
                    nc.scalar.activation(
                        scr[:, 0:2, :], ps[:, 2:4, :], ACTF.Square,
                        accum_out=accs[:, 1:2],
                    )

            nc.scalar.activation(
                scr[:, 0:2, 0:384], ps[:, 0:2, 0:384], ACTF.Square,
                accum_out=accs[:, 0:1],
            )
            nc.scalar.activation(
                scr[:, 0:2, :], ps[:, 4:6, :], ACTF.Square,
                accum_out=accs[:, 2:3],
            )
            nc.sync.dma_start(out=out, in_=accs)

    split_excess_waits(nc)
    return nc


_CACHE = {}


def _block_corrections(F):
    """sum_b ||F_b^T F_b||^2 and sum_b ||F_b^T F_{b+4 mod 8}||^2 over the 8
    column blocks (fp32, matching the device's fp8->fp32 Gram numerics)."""
    c0 = np.float64(0.0)
    c4 = np.float64(0.0)
    for b in range(NB):
        Fb = F[:, P * b : P * (b + 1)]
        b4 = (b + 4) % NB
        Fb4 = F[:, P * b4 : P * (b4 + 1)]
        c0 += np.float64(((Fb.T @ Fb) ** 2).sum(dtype=np.float64))
        c4 += np.float64(((Fb.T @ Fb4) ** 2).sum(dtype=np.float64))
    return c0, c4


def kernel(feat_q: np.ndarray, feat_k: np.ndarray) -> np.ndarray:
    import ml_dtypes

    fq = np.ascontiguousarray(np.asarray(feat_q, dtype=np.float32))
    fk = np.ascontiguousarray(np.asarray(feat_k, dtype=np.float32))
    assert fq.shape == (N, D) and fk.shape == (N, D)

    if "nc" not in _CACHE:
        _CACHE["nc"] = build_program()
    nc = _CACHE["nc"]

    s32 = np.float32(SCALE)
    qh = fq / np.linalg.norm(fq, axis=1, keepdims=True) * s32
    kh = fk / np.linalg.norm(fk, axis=1, keepdims=True) * s32
    q8 = qh.astype(ml_dtypes.float8_e4m3)
    k8 = kh.astype(ml_dtypes.float8_e4m3)

    in_maps = []
    for c in range(N_CORES):
        in_maps.append(
            {
                "sq": np.ascontiguousarray(np.roll(q8, -P * c, axis=1)[:, :QW]),
                "sk": np.ascontiguousarray(np.roll(k8, -P * c, axis=1)[:, :KW]),
            }
        )
    res = run_bass_kernel_spmd(nc, in_maps, list(range(N_CORES)))

    acc = np.zeros(3, dtype=np.float64)
    for c in range(N_CORES):
        acc += np.asarray(res.results[c]["out"], dtype=np.float64).sum(axis=0)
    acc_qqkk = acc[0]
    acc_qk = acc[1] + acc[2]

    qf = q8.astype(np.float32)
    kf = k8.astype(np.float32)
    c0q, c4q = _block_corrections(qf)
    c0k, c4k = _block_corrections(kf)

    S = 2.0 * acc_qqkk + c0q + c4q + c0k + c4k - 2.0 * acc_qk
    loss = S / (SCALE**4) / (N * (N - 1))
    return np.asarray(np.float32(loss))


if __name__ == "__main__":
    rng = np.random.default_rng(0)
    q = rng.standard_normal((N, D)).astype(np.float32)
    k = rng.standard_normal((N, D)).astype(np.float32)
    print("loss:", kernel(q, k))


# revision 9
# speedup vs baseline: 1.0108x; 1.0108x over previous
"""Trainium2 Bass kernel for nn_ConstLoss_22746146800082 (fp8 factorized).

loss = mean_{i != j} (Cq[i,j] - Ck[i,j])^2 with Cx the pairwise cosine matrix
of feat_x (N=4096, D=1024).  The Normalize/cosine eps terms cancel, so Cx is
the cosine matrix of the raw rows and the diagonal of Cq - Ck is ~0.

Factorization: ||Cq - Ck||_F^2 = ||Aqq||^2 + ||Akk||^2 - 2 ||Aqk||^2 with the
feature-space Grams Aqq = Qh^T Qh, Akk = Kh^T Kh, Aqk = Kh^T Qh of the
row-normalized features (1024x1024 each).

Host prep: rows are normalized, scaled by 32 and quantized to fp8e4 (e4m3) on
the host, so the device does no normalization, no collective, and the
stationary matmul operand is just a column slice of the streamed tile.

Sharding: core c owns feature block c (128 of 1024 output rows per Gram).
Each core's inputs have their columns ROTATED left by 128*c, so every core
runs the identical program.  sq ships Q blocks {c..c+3} (local [0:512)), sk
ships K blocks {c..c+4} (local [0:640)).  Per core: qq = Q_c^T Q_{c+1..c+3}
(block distances 1-3; summed over cores that covers every off-diagonal pair
except distance 4), kk likewise, and qk uses TWO K stationaries (K_c and
K_{c+4}) against the 4 Q moving blocks, which tiles all 64 (K_i, Q_j) block
pairs exactly once (j-i in {0..3} from K_c, {-4..-1} from K_{c+4}).  Matmuls
run as fp8 DoubleRow (2 chunks of 128 samples per instruction).  The device
returns raw per-region sums of squares; the host combines them as
2*acc + C0 + C4 with the diagonal-block and distance-4 Gram corrections
C0/C4 computed on the host from the same fp8 data (16% of the Gram FLOPs).
"""

import numpy as np

import concourse.bass as bass
import concourse.mybir as mybir
import concourse.tile as tile
from concourse.vector_clock import ScopedClock
from concourse.bass_utils import run_bass_kernel_spmd

N_CORES = 8
N = 4096
D = 1024
P = 128
NB = D // P               # 8 feature blocks
QW = 4 * P                # 512: sq ships Q blocks {c..c+3}
KW = 5 * P                # 640: sk ships K blocks {c..c+4}
MG = 4                    # chunks per DMA
ND = N // (P * MG)        # 8 DMA iterations
NG = N // (2 * P)         # 16 DoubleRow groups (2 chunks each)
GEN = NG // 2             # qk psum generation split
SCALE = 32.0

F32 = mybir.dt.float32
F8 = mybir.dt.float8e4
ACTF = mybir.ActivationFunctionType
DR = mybir.MatmulPerfMode.DoubleRow


class _TC(tile.TileContext):
    """TileContext whose kernel-tail drain splits its semaphore waits across
    preceding sync-engine NOPs: this container's walrus build rejects a Drain
    carrying more than one sync wait ("Too many sync wait commands")."""

    def _drain_and_barrier(self, tick_clock, wait_clock):
        nc = self.nc
        probe = nc.sync.nop(nofuse=True)
        wait_clock.add_sem_waits(
            probe.ins, ScopedClock({None: tick_clock.global_clock})
        )
        waits = list(probe.ins.sync_info.on_wait or []) if probe.ins.sync_info else []
        if probe.ins.sync_info is not None:
            probe.ins.sync_info.on_wait = waits[:1]
        for w in waits[1:]:
            n2 = nc.sync.nop(nofuse=True)
            n2.ins.sync_info = mybir.SyncInfo(on_wait=[w], on_update=[])
        nc.sync.drain()
        nc.all_engine_barrier()
        popped = nc._tile_sem_poison_stack.pop()
        assert popped is self._sem_poison
        nc.clear_and_free_semaphores(list(self.sems.allocated().values()))
        nc.all_engine_barrier()


MAX_WAITS_PER_INST = 1


def split_excess_waits(nc):
    """walrus (this build) rejects instructions carrying more than a couple
    of semaphore waits.  Hoist excess waits onto injected same-engine NOPs
    placed immediately before the offending instruction."""
    n = 0
    for f in nc.m.functions:
        for bb in f.blocks:
            insts = bb.instructions
            out = []
            changed = False
            for ins in insts:
                si = ins.sync_info
                waits = list(si.on_wait or []) if si is not None else []
                while len(waits) > MAX_WAITS_PER_INST:
                    take = waits[:MAX_WAITS_PER_INST]
                    waits = waits[MAX_WAITS_PER_INST:]
                    nop = mybir.InstNoOp(name=f"I-waitsplit-{n}", ins=[], outs=[])
                    n += 1
                    nop.engine = ins.engine
                    nop.sync_info = mybir.SyncInfo(on_wait=take, on_update=[])
                    out.append(nop)
                    changed = True
                if changed and si is not None:
                    si.on_wait = waits
                out.append(ins)
            if changed:
                bb.instructions = out
    return n


def build_program(sim_mode: bool = False):
    nc = bass.Bass(
        "TRN2", target_bir_lowering=False, debug=False, num_devices=N_CORES
    )
    sq = nc.dram_tensor("sq", [N, QW], F8, kind="ExternalInput").ap()
    sk = nc.dram_tensor("sk", [N, KW], F8, kind="ExternalInput").ap()
    out = nc.dram_tensor("out", [P, 3], F32, kind="ExternalOutput").ap()

    with _TC(nc) as tc:
        with (
            tc.tile_pool(name="stream", bufs=3) as stream,
            tc.tile_pool(name="fin", bufs=1) as fin,
            tc.tile_pool(name="psum", bufs=1, space="PSUM") as psum,
        ):
            accs = fin.tile([P, 3], F32)
            scr = fin.tile([P, 2, 512], F32)

            # One psum tile covering 6 of 8 banks, viewed as [P, bank, 512]:
            # bank 0 = qq [0:384), bank 1 = kk [0:384), banks 2-3 = qk
            # generation A (K_c and K_{c+4} stationary rows), banks 4-5 = qk
            # generation B.  Every accumulation region sits alone in its bank
            # so the whole-bank start_tensor_calc zeroing is safe.
            ps = psum.tile([P, 6, 512], F32, name="ps", tag="ps")

            # chunk schedule: 4-chunk DMAs mid-stream, 2-chunk (single-group)
            # DMAs at the end so the final transfer gates only one group of
            # matmuls
            sched = [4] * (2 * NG // 4)
            g0 = 0
            for mg in sched:
                c0 = 2 * g0  # first chunk of this DMA
                tq = stream.tile([P, mg, QW], F8, name="tq", tag="tq")
                tk = stream.tile([P, mg, KW], F8, name="tk", tag="tk")
                nc.sync.dma_start(
                    out=tk,
                    in_=bass.AP(
                        sk.tensor, sk.offset + c0 * P * KW,
                        [[KW, P], [P * KW, mg], [1, KW]],
                    ),
                )
                nc.sync.dma_start(
                    out=tq,
                    in_=bass.AP(
                        sq.tensor, sq.offset + c0 * P * QW,
                        [[QW, P], [P * QW, mg], [1, QW]],
                    ),
                )
                for h in range(mg // 2):
                    g = g0 + h
                    sl = slice(2 * h, 2 * h + 2)
                    lq = tq[:, sl, 0:P]      # [128, 2, 128] stationary Q_c
                    lk0 = tk[:, sl, 0:P]     # [128, 2, 128] stationary K_c
                    lk4 = tk[:, sl, 4 * P : 5 * P]  # stationary K_{c+4}
                    st = dict(
                        start=(g == 0), stop=(g == NG - 1),
                        perf_mode=DR, skip_group_check=True,
                    )
                    nc.tensor.matmul(ps[:, 1, 0:384], lhsT=lk0, rhs=tk[:, sl, P:QW], **st)
                    nc.tensor.matmul(ps[:, 0, 0:384], lhsT=lq, rhs=tq[:, sl, P:QW], **st)
                    qb = 2 if g < GEN else 4
                    stq = dict(
                        start=(g % GEN == 0), stop=(g % GEN == GEN - 1),
                        perf_mode=DR, skip_group_check=True,
                    )
                    nc.tensor.matmul(ps[:, qb, :], lhsT=lk0, rhs=tq[:, sl, 0:QW], **stq)
                    nc.tensor.matmul(ps[:, qb + 1, :], lhsT=lk4, rhs=tq[:, sl, 0:QW], **stq)
                g0 += mg // 2

                if g0 == GEN:
                    # qk generation A done: square it while the second half
                    # of the stream is still in flight
                    nc.scalar.activation(
                        scr[:, 0:2, :], ps[:, 2:4, :], ACTF.Square,
                        accum_out=accs[:, 1:2],
                    )

            nc.scalar.activation(
                scr[:, 0:2, 0:384], ps[:, 0:2, 0:384], ACTF.Square,
                accum_out=accs[:, 0:1],
            )
            nc.scalar.activation(
                scr[:, 0:2, :], ps[:, 4:6, :], ACTF.Square,
                accum_out=accs[:, 2:3],
            )
            nc.sync.dma_start(out=out, in_=accs)

    split_excess_waits(nc)
    return nc


_CACHE = {}


def _block_corrections(F):
    """sum_b ||F_b^T F_b||^2 and sum_b ||F_b^T F_{b+4 mod 8}||^2 over the 8
    column blocks (fp32, matching the device's fp8->fp32 Gram numerics)."""
    c0 = np.float64(0.0)
    c4 = np.float64(0.0)
    for b in range(NB):
        Fb = F[:, P * b : P * (b + 1)]
        b4 = (b + 4) % NB
        Fb4 = F[:, P * b4 : P * (b4 + 1)]
        c0 += np.float64(((Fb.T @ Fb) ** 2).sum(dtype=np.float64))
        c4 += np.float64(((Fb.T @ Fb4) ** 2).sum(dtype=np.float64))
    return c0, c4


def kernel(feat_q: np.ndarray, feat_k: np.ndarray) -> np.ndarray:
    import ml_dtypes

    fq = np.ascontiguousarray(np.asarray(feat_q, dtype=np.float32))
    fk = np.ascontiguousarray(np.asarray(feat_k, dtype=np.float32))
    assert fq.shape == (N, D) and fk.shape == (N, D)

    if "nc" not in _CACHE:
        _CACHE["nc"] = build_program()
    nc = _CACHE["nc"]

    s32 = np.float32(SCALE)
    qh = fq / np.linalg.norm(fq, axis=1, keepdims=True) * s32
    kh = fk / np.linalg.norm(fk, axis=1, keepdims=True) * s32
    q8 = qh.astype(ml_dtypes.float8_e4m3)
    k8 = kh.astype(ml_dtypes.float8_e4m3)

    in_maps = []
    for c in range(N_CORES):
        in_maps.append(
            {
                "sq": np.ascontiguousarray(np.roll(q8, -P * c, axis=1)[:, :QW]),
                "sk": np.ascontiguousarray(np.roll(k8, -P * c, axis=1)[:, :KW]),
            }
        )
    res = run_bass_kernel_spmd(nc, in_maps, list(range(N_CORES)))

    acc = np.zeros(3, dtype=np.float64)
    for c in range(N_CORES):
        acc += np.asarray(res.results[c]["out"], dtype=np.float64).sum(axis=0)
    acc_qqkk = acc[0]
    acc_qk = acc[1] + acc[2]

    qf = q8.astype(np.float32)
    kf = k8.astype(np.float32)
    c0q, c4q = _block_corrections(qf)
    c0k, c4k = _block_corrections(kf)

    S = 2.0 * acc_qqkk + c0q + c4q + c0k + c4k - 2.0 * acc_qk
    loss = S / (SCALE**4) / (N * (N - 1))
    return np.asarray(np.float32(loss))


if __name__ == "__main__":
    rng = np.random.default_rng(0)
    q = rng.standard_normal((N, D)).astype(np.float32)
    k = rng.standard_normal((N, D)).astype(np.float32)
    print("loss:", kernel(q, k))
